# revision 1
# baseline (speedup 1.0000x reference)
"""Trainium2 Bass kernel for the MnnCoreModule activation functions.

Strategy: all four expensive quadrature-defined univariate functions
(G, H_neg, H_pos, and the erfcx inside g) are replaced by low-degree
polynomial fits in transformed variables (v = 1/(1-x) on the negative
side, y = 1/(1+|d|) for erfcx, plain x for the positive-side ratios
G*e^(-x^2)/x and H*e^(-2x^2)/x).  Everything on the positive side
shares one ACT Exp of d^2 (d = max(x,-3)); all fitted positive-side
parts vanish at x=0 so negative/positive branches combine additively
with no selects.  sqrt/rsqrt/divide go through exp/ln on the Scalar
engine or exact DVE reciprocal.  Elementwise work runs on the Vector
engine; affines/transcendentals on the Scalar engine.

Sharding: purely elementwise; the [128,1024] inputs are split into 8
column slices of [128,128], one per NeuronCore; outputs are
concatenated back.
"""
import math
import os
import numpy as np
from contextlib import ExitStack

import concourse.bass as bass
import concourse.tile as tile
import concourse.mybir as mybir
from concourse import bacc
from concourse.bass_utils import run_bass_kernel_spmd

F32 = mybir.dt.float32
ALU = mybir.AluOpType
ACT = mybir.ActivationFunctionType

H = 128          # half (per-point) width
W = 2 * H        # stacked [ub | lb] width
P = 128          # partitions
N_CORES = 8

SL = math.sqrt(0.05)
CUTSL = 10.0 * SL
C_G = 0.8862269254527580          # sqrt(pi)/2
CHI_C = 2.0 / 0.05 ** 1.5         # 178.885...

# ---- polynomial fits ----
# A_G, A_H: monomial in raw v = 1/(1-x);  A_E: monomial in raw y = 1/(1+|d|)
# (A_E is pre-scaled by C_G);  A_PG/A_PH: monomial in t = s*x + b.
A_G = [-0.8374120249939152, 1.000124051786955, -0.1250860686309572, 0.0823141551932803, -0.32553977908767595, 0.2931065046756127, -0.08752734654561047]
A_H = [-0.15419699857454666, -0.0005765328570554451, 0.0718612655040626, 0.04876864475248427, 0.45392174474643154, -0.9406922275990531, 0.7976458022400215, -0.33339648059720967, 0.05666398452993321]
A_E = [-0.00032720070804087564, 0.5079718623271994, 0.4207241381073852, 0.6620984990090895, -1.4179378889505805, 0.9474401317532978, -0.23377118188596566]
A_PG = [0.11505177758939311, -0.2512577996191818, 0.4275136232890099, -0.6482704305023361, 0.8294620965311406, -0.7658724278017492, 0.24873644444397702, 1.0938578370455956, -3.264450685298945, 3.2600218388791213, 1.9031162249540459, -5.933893040612396, 1.60184163990094, 3.616118179615748, -1.9693917648272503, -0.800260599053927, 0.5648145323042078]
s_PG = 0.34722222825038596
b_PG = -1.0000000347222229
A_PH = [0.019044601681638518, -0.06386582004476829, 0.1464577105098912, -0.27877120834740665, 0.44394665619111057, -0.5199481026610917, 0.2945063988707525, 0.12518752045700438, -0.36639308279423777, 1.2151723410967217, -3.114151021006391, 1.9850080838628634, 3.972081673384112, -5.872540390456061, -0.41605790805653176, 4.306321574819455, -1.337250789410191, -1.0490599995340013, 0.5124325471259535]
s_PH = 0.34722222825038596
b_PH = -1.0000000347222229

_NC_CACHE = {}
last_exec_time_ns = None
last_results = None


def _poly_chain(nc, pool, t_ap, coeffs, name, out_ap=None, drop_a0=False):
    """Evaluate ascending-coeff polynomial at tile AP t_ap via the
    (acc + c) * t STT chain.  Returns (tile, pending_a0):
    if drop_a0, a0 is never added (caller's combine must absorb/cancel it);
    otherwise returns the chain missing a0 and the a0 value (caller folds it).
    """
    d = len(coeffs) - 1
    acc = pool.tile([P, t_ap.shape[1]], F32, name=f"{name}_acc", tag=f"{name}_acc")
    acc2 = pool.tile([P, t_ap.shape[1]], F32, name=f"{name}_acc2", tag=f"{name}_acc2")
    # init: acc = t*c[d] + c[d-1]
    nc.vector.tensor_scalar(acc[:], t_ap, float(coeffs[d]), float(coeffs[d - 1]),
                            ALU.mult, ALU.add)
    # merge slot then c[d-2] .. c[1]; ping-pong buffers (no in-place RAW)
    consts = [0.0] + [float(c) for c in coeffs[d - 2:0:-1]]
    cur, nxt = acc, acc2
    for cc in consts:
        nc.vector.scalar_tensor_tensor(nxt[:], cur[:], cc, t_ap, ALU.add, ALU.mult)
        cur, nxt = nxt, cur
    return cur, float(coeffs[0])


def _poly_chain_estrin(nc, pool, tc_pool_T, t_ap, t2_ap, coeffs, name):
    """p(t) - a0 via even/odd split: A(t^2)-a0 + t*B(t^2).
    t2_ap must hold t*t.  Returns (tile, a0)."""
    a_even = [float(c) for c in coeffs[0::2]]
    a_odd = [float(c) for c in coeffs[1::2]]
    Ach, aA0 = _poly_chain(nc, pool, t2_ap, a_even, f"{name}_ev")
    Bch, aB0 = _poly_chain(nc, pool, t2_ap, a_odd, f"{name}_od")
    tB = pool.tile([P, t_ap.shape[1]], F32, name=f"{name}_tb", tag=f"{name}_tb")
    nc.vector.scalar_tensor_tensor(tB[:], Bch[:], aB0, t_ap, ALU.add, ALU.mult)
    out = pool.tile([P, t_ap.shape[1]], F32, name=f"{name}_es", tag=f"{name}_es")
    nc.vector.tensor_add(out[:], Ach[:], tB[:])
    return out, aA0


def _build(trace_unused=False):
    nc = bacc.Bacc("TRN2", target_bir_lowering=False, debug=False,
                   num_devices=N_CORES)
    u_d = nc.dram_tensor("u", [P, H], F32, kind="ExternalInput")
    s_d = nc.dram_tensor("s", [P, H], F32, kind="ExternalInput")
    ua_d = nc.dram_tensor("ua", [P, H], F32, kind="ExternalOutput")
    sa_d = nc.dram_tensor("sa", [P, H], F32, kind="ExternalOutput")
    chi_d = nc.dram_tensor("chi", [P, H], F32, kind="ExternalOutput")

    with tile.TileContext(nc) as tc, ExitStack() as ctx:
        pool = ctx.enter_context(tc.tile_pool(name="p", bufs=1))

        def T(name, w=H):
            return pool.tile([P, w], F32, name=name, tag=name)

        u = T("u_t"); s = T("s_t")
        nc.gpsimd.dma_start(u[:], u_d.ap())
        nc.gpsimd.dma_start(s[:], s_d.ap())

        # ---------------- setup / masks ----------------
        m1 = T("m1")
        nc.vector.tensor_single_scalar(m1[:], s[:], 0.0, ALU.is_gt)
        wneg = T("wneg")                                   # u - 1 (exact)
        nc.vector.tensor_single_scalar(wneg[:], u[:], 1.0, ALU.subtract)
        t1 = T("t1")                                       # CUT*SL*s
        nc.vector.tensor_single_scalar(t1[:], s[:], CUTSL, ALU.mult)
        t2 = T("t2")
        nc.gpsimd.tensor_add(t2[:], t1[:], wneg[:])
        mra = T("mra")
        nc.vector.tensor_single_scalar(mra[:], t2[:], 0.0, ALU.is_gt)
        reg1 = T("reg1")
        nc.gpsimd.tensor_mul(reg1[:], mra[:], m1[:])
        reg0 = T("reg0")
        nc.vector.tensor_scalar(reg0[:], m1[:], -1.0, 1.0, ALU.mult, ALU.add)
        mu1 = T("mu1")
        nc.vector.tensor_single_scalar(mu1[:], u[:], 1.0, ALU.is_gt)
        reg2 = T("reg2")
        nc.gpsimd.tensor_mul(reg2[:], reg0[:], mu1[:])
        # region2 path (independent of the point block; runs early on ACT)
        UU2 = T("UU2")
        nc.vector.scalar_tensor_tensor(UU2[:], u[:], 2.0, reg2[:],
                                       ALU.subtract, ALU.mult)
        nc.vector.tensor_scalar_add(UU2[:], UU2[:], 2.0)
        RU = T("RU")
        nc.vector.reciprocal(RU[:], UU2[:])                # exact 1/u2
        OMU = T("OMU")
        nc.vector.tensor_scalar(OMU[:], RU[:], -1.0, 1.0, ALU.mult, ALU.add)
        LNOMU = T("LNOMU")
        nc.scalar.activation(LNOMU[:], OMU[:], ACT.Ln)
        LOGT = T("LOGT")
        nc.scalar.activation(LOGT[:], LNOMU[:], ACT.Copy, bias=5.0, scale=-20.0)
        L2 = T("L2")
        nc.scalar.activation(L2[:], LOGT[:], ACT.Ln)
        UA2 = T("UA2")
        nc.scalar.activation(UA2[:], L2[:], ACT.Exp, bias=0.0, scale=-1.0)
        TQ = T("TQ")                                       # 2u - 1
        nc.scalar.activation(TQ[:], u[:], ACT.Copy, bias=-1.0, scale=2.0)
        TZ = T("TZ")
        nc.gpsimd.tensor_mul(TZ[:], TQ[:], LOGT[:])
        L3 = T("L3")
        nc.scalar.activation(L3[:], TZ[:], ACT.Ln, bias=0.0, scale=1.0 / 40.0)
        CHI2 = T("CHI2")
        nc.scalar.activation(CHI2[:], L3[:], ACT.Exp, bias=0.0, scale=-0.5)
        CHI2M = T("CHI2M")
        nc.gpsimd.tensor_mul(CHI2M[:], CHI2[:], reg2[:])

        # s_safe = s + (s<=0);  1/SL folded into the numerators (ACT, off
        # the critical path);  exact reciprocal of s_safe on DVE.
        m0 = T("m0")
        nc.vector.tensor_single_scalar(m0[:], s[:], 0.0, ALU.is_le)
        q = T("q")
        nc.gpsimd.tensor_add(q[:], s[:], m0[:])
        rq = T("rq")
        nc.vector.reciprocal(rq[:], q[:])
        wsl = T("wsl")                                     # (1-u)/SL
        nc.vector.tensor_scalar(wsl[:], u[:], -1.0 / SL, 1.0 / SL, ALU.mult, ALU.add)
        usl = T("usl")                                     # -u/SL
        nc.vector.tensor_single_scalar(usl[:], u[:], -1.0 / SL, ALU.mult)

        X = T("X", W)                                      # [ub | lb]
        nc.vector.tensor_mul(X[:, 0:H], wsl[:], rq[:])
        nc.vector.tensor_mul(X[:, H:W], usl[:], rq[:])

        # ---------------- stacked point block ----------------
        D = T("D", W)
        nc.vector.tensor_single_scalar(D[:], X[:], -3.0, ALU.max)
        D2 = T("D2", W)
        nc.scalar.activation(D2[:], D[:], ACT.Square)
        ED2 = T("ED2", W)
        nc.scalar.activation(ED2[:], D2[:], ACT.Exp)
        MDIR = T("MDIR", W)
        nc.vector.tensor_single_scalar(MDIR[:], X[:], -3.0, ALU.is_ge)
        MPOS = T("MPOS", W)
        nc.vector.tensor_single_scalar(MPOS[:], X[:], 0.0, ALU.is_ge)
        XM = T("XM", W)
        nc.vector.tensor_single_scalar(XM[:], X[:], 0.0, ALU.min)
        OMX = T("OMX", W)                                  # 1 - xm
        nc.vector.tensor_scalar(OMX[:], XM[:], -1.0, 1.0, ALU.mult, ALU.add)

        def pos_recip(src, name):
            """1/src via the exact (correctly-rounded) DVE reciprocal."""
            out = T(f"{name}_r", src.shape[1])
            nc.vector.reciprocal(out[:], src[:])
            return out

        RV = pos_recip(OMX, "v")                           # 1/(1-xm)

        # G_neg = qG(v) - 0.5*ln(1 - xm/2)   (a0 dropped; cancels in dG)
        Gchain, _ = _poly_chain(nc, pool, RV[:], A_G, "G")
        LNV = T("LNV", W)
        nc.scalar.activation(LNV[:], XM[:], ACT.Ln, bias=1.0, scale=-0.5)
        GN = T("GN", W)
        nc.vector.scalar_tensor_tensor(GN[:], LNV[:], -0.5, Gchain[:],
                                       ALU.mult, ALU.add)

        # H_neg  (a0 dropped; cancels in dH)
        HN, _ = _poly_chain(nc, pool, RV[:], A_H, "Hn")

        # ---------------- ub-only positive side ----------------
        XP = T("XP")
        nc.vector.tensor_single_scalar(XP[:], X[:, 0:H], 0.0, ALU.max)
        TP = T("TP")
        nc.scalar.activation(TP[:], XP[:], ACT.Copy, bias=b_PG, scale=s_PG)
        TPH = T("TPH")
        nc.scalar.activation(TPH[:], XP[:], ACT.Copy, bias=b_PH, scale=s_PH)
        PGc, pg0 = _poly_chain(nc, pool, TP[:], A_PG, "PG")
        MG = T("MG")
        nc.vector.scalar_tensor_tensor(MG[:], PGc[:], pg0, XP[:], ALU.add, ALU.mult)
        GPOS = T("GPOS")
        nc.gpsimd.tensor_mul(GPOS[:], MG[:], ED2[:, 0:H])
        PHc, ph0 = _poly_chain(nc, pool, TPH[:], A_PH, "PH")
        MH = T("MH")
        nc.vector.scalar_tensor_tensor(MH[:], PHc[:], ph0, XP[:], ALU.add, ALU.mult)
        ED4 = T("ED4")
        nc.scalar.activation(ED4[:], ED2[:, 0:H], ACT.Square)
        HPOS = T("HPOS")
        nc.gpsimd.tensor_mul(HPOS[:], MH[:], ED4[:])

        # ---------------- combine ----------------
        dG = T("dG")
        nc.gpsimd.tensor_sub(dG[:], GN[:, 0:H], GN[:, H:W])
        nc.gpsimd.tensor_add(dG[:], dG[:], GPOS[:])
        dH = T("dH")
        nc.gpsimd.tensor_sub(dH[:], HN[:, 0:H], HN[:, H:W])
        nc.gpsimd.tensor_add(dH[:], dH[:], HPOS[:])

        # erfcx argument tile: ub half = 1/(1+|d|) (fresh sw recip); lb half
        # reuses RV (equal where the E value is used; elsewhere masked off).
        RY = T("RY", W)
        ABSD = T("ABSD")
        nc.scalar.activation(ABSD[:], D[:, 0:H], ACT.Abs)
        YA = T("YA")                                       # 1 + |d| (ub)
        nc.scalar.activation(YA[:], ABSD[:], ACT.Copy, bias=1.0, scale=1.0)
        nc.vector.reciprocal(RY[:, 0:H], YA[:])
        nc.scalar.activation(RY[:, H:W], RV[:, H:W], ACT.Copy)

        Z = T("Z", W)                                      # max(-x, 3)
        nc.vector.tensor_scalar(Z[:], X[:], -1.0, 3.0, ALU.mult, ALU.max)
        WZ = pos_recip(Z, "wz")                            # 1/z
        W2 = T("W2", W)
        nc.scalar.activation(W2[:], WZ[:], ACT.Square)

        # asym:  0.5/z * (1 - .5 w2 + .75 w2^2 - 1.875 w2^3)
        #      = wz * (0.5 - 0.25 w2 + 0.375 w2^2 - 0.9375 w2^3)
        aa = T("aa", W)
        aa2 = T("aa2", W)
        nc.vector.tensor_scalar(aa[:], W2[:], -0.9375, 0.375, ALU.mult, ALU.add)
        nc.vector.scalar_tensor_tensor(aa2[:], aa[:], 0.0, W2[:], ALU.add, ALU.mult)
        nc.vector.scalar_tensor_tensor(aa[:], aa2[:], -0.25, W2[:], ALU.add, ALU.mult)
        GASYM = T("GASYM", W)
        nc.vector.scalar_tensor_tensor(GASYM[:], aa[:], 0.5, WZ[:],
                                       ALU.add, ALU.mult)

        # erfcx (C-scaled) -> direct-branch g
        Echain, e0 = _poly_chain(nc, pool, RY[:], A_E, "E")
        SIG = T("SIG", W)                                  # 1 - 2*mpos
        nc.scalar.activation(SIG[:], MPOS[:], ACT.Copy, bias=1.0, scale=-2.0)
        TSG = T("TSG", W)
        nc.vector.scalar_tensor_tensor(TSG[:], Echain[:], e0, SIG[:],
                                       ALU.add, ALU.mult)
        ED2M = T("ED2M", W)
        nc.gpsimd.tensor_mul(ED2M[:], ED2[:], MPOS[:])
        GDIR = T("GDIR", W)
        nc.vector.scalar_tensor_tensor(GDIR[:], ED2M[:], 2.0 * C_G, TSG[:],
                                       ALU.mult, ALU.add)
        GDIFF = T("GDIFF", W)
        nc.gpsimd.tensor_sub(GDIFF[:], GDIR[:], GASYM[:])
        GG = T("GG", W)                                    # g at both points
        nc.vector.scalar_tensor_tensor(GG[:], GDIFF[:], 0.0, MDIR[:],
                                       ALU.add, ALU.mult)
        nc.gpsimd.tensor_add(GG[:], GG[:], GASYM[:])

        dg = T("dg")
        nc.gpsimd.tensor_sub(dg[:], GG[:, 0:H], GG[:, H:W])
        DEN = T("DEN")
        nc.vector.tensor_scalar(DEN[:], dG[:], 40.0, 5.0, ALU.mult, ALU.add)
        UA1 = T("UA1")
        nc.vector.reciprocal(UA1[:], DEN[:])

        UAF = T("UAF")
        nc.gpsimd.tensor_mul(UAF[:], UA1[:], reg1[:])
        UA2M = T("UA2M")
        nc.gpsimd.tensor_mul(UA2M[:], UA2[:], reg2[:])
        nc.vector.tensor_add(UAF[:], UAF[:], UA2M[:])
        nc.sync.dma_start(ua_d.ap(), UAF[:])

        # s_a / chi paths use the unmasked u_a1 (valid on reg1; masked at the
        # end), keeping the final-output chain short.
        UASQ = T("UASQ")
        nc.vector.tensor_mul(UASQ[:], UA1[:], UA1[:])
        UA3 = T("UA3")
        nc.vector.tensor_mul(UA3[:], UASQ[:], UA1[:])
        T7 = T("T7")
        nc.vector.tensor_mul(T7[:], dH[:], UA3[:])
        nc.vector.tensor_single_scalar(T7[:], T7[:], 1e-30, ALU.max)
        LNVAL = T("LNVAL")
        nc.scalar.activation(LNVAL[:], T7[:], ACT.Ln, bias=0.0, scale=3200.0)
        SA0 = T("SA0")
        nc.scalar.activation(SA0[:], LNVAL[:], ACT.Exp, bias=0.0, scale=0.5)
        RSA = T("RSA")
        nc.scalar.activation(RSA[:], LNVAL[:], ACT.Exp, bias=0.0, scale=-0.5)
        SAF = T("SAF")
        nc.vector.tensor_mul(SAF[:], SA0[:], reg1[:])
        nc.sync.dma_start(sa_d.ap(), SAF[:])

        T8 = T("T8")
        nc.vector.tensor_mul(T8[:], UASQ[:], dg[:])
        T9 = T("T9")
        nc.vector.tensor_mul(T9[:], T8[:], RSA[:])
        CHI1M = T("CHI1M")
        nc.vector.scalar_tensor_tensor(CHI1M[:], T9[:], CHI_C, reg1[:],
                                       ALU.mult, ALU.mult)
        CHIF = T("CHIF")
        nc.vector.tensor_add(CHIF[:], CHI1M[:], CHI2M[:])
        nc.sync.dma_start(chi_d.ap(), CHIF[:])

    nc.finalize()
    _fix_act_tables(nc)
    return nc


def _fix_act_tables(nc):
    """Collapse the greedy exp_and_others/natural_log table-load thrash into
    one load of natural_log_exp_and_others (superset of every ACT function
    this kernel uses).  All loads are emitted sync-free, so dropping the
    redundant ones is safe."""
    from concourse.hw_specs import get_activation_tables
    tables = list(get_activation_tables(nc.m.arch).keys())
    target = tables.index("natural_log_exp_and_others")
    for b in nc.m.functions[0].blocks:
        keep_done = False
        removed = []
        for i in b.instructions:
            if isinstance(i, mybir.InstLoadActFuncSet):
                assert i.sync_info is None
                if not keep_done:
                    i.act_func_set_id = target
                    keep_done = True
                else:
                    removed.append(i)
        for i in removed:
            b.instructions.remove(i)


def kernel(u: np.ndarray, s: np.ndarray):
    global last_exec_time_ns, last_results
    u = np.ascontiguousarray(np.asarray(u, dtype=np.float32))
    s = np.ascontiguousarray(np.asarray(s, dtype=np.float32))
    assert u.shape == (P, N_CORES * H) and s.shape == (P, N_CORES * H)

    if "nc" not in _NC_CACHE:
        _NC_CACHE["nc"] = _build()
    nc = _NC_CACHE["nc"]

    in_maps = []
    for i in range(N_CORES):
        sl = np.s_[:, i * H:(i + 1) * H]
        in_maps.append({"u": np.ascontiguousarray(u[sl]),
                        "s": np.ascontiguousarray(s[sl])})

    res = run_bass_kernel_spmd(nc, in_maps, list(range(N_CORES)))
    last_exec_time_ns = res.exec_time_ns
    last_results = res

    ua = np.empty((P, N_CORES * H), np.float32)
    sa = np.empty((P, N_CORES * H), np.float32)
    chi = np.empty((P, N_CORES * H), np.float32)
    for i, r in enumerate(res.results):
        sl = np.s_[:, i * H:(i + 1) * H]
        ua[sl] = r["ua"]
        sa[sl] = r["sa"]
        chi[sl] = r["chi"]
    return ua, sa, chi

